# revision 8
# baseline (speedup 1.0000x reference)
"""Trainium2 Bass kernel for the eigenvalue/eigenvector loss
(nn_AV_loss): per-voxel 3x3 symmetric eigendecomposition of input and
target tensors, masked L1 of sorted eigenvalues + masked principal-axis
|cosine|, reduced to two scalars.

Self-contained: hardcodes shapes/sharding. kernel(**inputs) takes FULL
inputs and returns the full output (val_loss, vec_loss).

Sharding: fully data-parallel over B*H (2*80 = 160 -> 20 H-slices per
core); per-core partial masked sums are returned and reduced on host.

v3 design (from the 54.5us v2):
- 6 device channels [d,e,f,aq] + [cq,bq] (q no longer shipped; host
  ships dq = q_in - q_tgt as a half plane) in bf16, halved DMA pieces
  so compute starts ~2us earlier.
- p2 via the zero-trace identity with aq^2 from the ACT square batch:
  p2' = (d^2+e^2+f^2+aq^2) - bq*cq  (kills 3 DVE adds).
- merged 2-wide DVE ops (s-pairs, bq*sqe|cq*sqd, e*b1|f*a1,
  df-m2|de-m4) to amortize per-op overhead.
- eigenvalue-diff tail in difference space: dl1 = dpc1+dq etc., abs via
  tensor_scalar(abs_max, 0) with free accum (no ACT Abs, no q planes).
- bf16 "Quake" rsqrt on DVE (2 uint16 tensor_scalar + 1 Newton step)
  for the eigvec normalization: kills the third ACT table load and the
  DVE iterative reciprocal.
- gpsimd (Pool) takes the dpc1/dpc3n/tsum branch (otherwise idle).
- input|target half-split software pipelining through the whole spine
  so ACT ladder overlaps the DVE det chain of the other half.
- 2 ACT table loads total: natural_log_exp -> trig_and_small.

Masked-voxel compaction: host packs ALL masked voxels of a core
row-major into [128, CW] (pad slots get a benign diag(1,2,3) matrix
that adds exactly 0 to the eigenvalue-L1 sum and exactly 1 per pad to
the |dot| sum, subtracted on host).
"""

import numpy as np
import ml_dtypes

import concourse.tile as tile
from concourse import mybir
from concourse.bacc import Bacc
from concourse.bass_utils import run_bass_kernel_spmd
from bass_rust import add_dep_helper


class _CapacityError(RuntimeError):
    pass


AF = mybir.ActivationFunctionType
OP = mybir.AluOpType
F32 = mybir.dt.float32
BF16 = mybir.dt.bfloat16
U16 = mybir.dt.uint16

NCORES = 8
B, C, H, W, D = 2, 6, 80, 80, 80
HS = H // (NCORES // B)          # 20 h-slices per core
P = 128
CW = 504                         # compact width (max masked 64269 <= 64512)
PK = 2 * CW                      # packed cols: [input | target]

TBL_LNEXP = 6                    # natural_log_exp_and_others
TBL_TRIG = 9                     # trig_and_small (arctan + sin)
TBL_SQRT = 3                     # sqrt_and_others (fallback tail only)
QUAKE = False                     # DVE quake-rsqrt tail vs ACT sqrt tail

# benign pad matrix diag(1,2,3): q=2, aq=-1, bq=0, cq=1, d=e=f=0
# lam={3,2,1}, input==target so d|lam|=0 and |cos|=1 per pad
PAD_CH = (0.0, 0.0, 0.0, -1.0, 1.0, 0.0)   # d,e,f,aq,cq,bq

CLAMP = 1.0 - 3e-7
PI2 = float(np.pi / 2.0)
MPI6 = float(-np.pi / 6.0)
LN4 = float(np.log(4.0))
MAGIC = float(0x5F37)            # bf16 quake-rsqrt magic (top16 of 0x5f3759df)


def _build():
    nc = Bacc()
    x = nc.dram_tensor("x", [6, P, PK], BF16, kind="ExternalInput")
    dqx = nc.dram_tensor("dqx", [P, CW], BF16, kind="ExternalInput")
    out = nc.dram_tensor("out", [P, 2], F32, kind="ExternalOutput")

    def tload(set_id, name):
        raw = mybir.InstLoadActFuncSet(
            name=name, ins=[], outs=[], act_func_set_id=set_id)
        nc.scalar.add_instruction(raw)
        return raw

    HH = (slice(0, CW), slice(CW, PK))

    with tile.TileContext(nc) as tc:
        with tc.tile_pool(name="main", bufs=1) as pool:

            def T(tag, shape=None, dt=BF16):
                return pool.tile(shape or [P, PK], dt, tag=tag, name=tag)

            out_sb = pool.tile([P, 2], F32, tag="out_sb")
            c05 = pool.tile([P, 1], F32, tag="c05")
            nc.vector.memset(c05, 0.5)
            pi2c = pool.tile([P, 1], F32, tag="pi2c")
            nc.vector.memset(pi2c, PI2)
            mpi6c = pool.tile([P, 1], F32, tag="mpi6c")
            nc.vector.memset(mpi6c, MPI6)
            ln4c = pool.tile([P, 1], F32, tag="ln4c")
            nc.vector.memset(ln4c, LN4)

            tl_a = tload(TBL_LNEXP, "tl_lnexp")

            # ---- loads: T1 = [d,e,f,aq], T2 = [cq,bq], dq ----
            T1 = T("T1", [P, 4, PK])
            T2 = T("T2", [P, 2, PK])
            dqt = T("dqt", [P, CW])
            for hs in HH:
                for i in range(4):
                    nc.sync.dma_start(out=T1[:, i, hs], in_=x[i, :, hs])
                for i in range(2):
                    nc.sync.dma_start(out=T2[:, i, hs], in_=x[4 + i, :, hs])
            nc.sync.dma_start(out=dqt, in_=dqx[:, :])

            # ---- ACT: squares of [d,e,f,aq] per half ----
            sq4 = T("sq4", [P, 4, PK])
            for hs in HH:
                a_sq = nc.scalar.activation(out=sq4[:, :, hs],
                                            in_=T1[:, :, hs],
                                            func=AF.Square)
                add_dep_helper(a_sq.ins, tl_a, False, "sq after tbl")
            sqd = sq4[:, 0, :]
            sqe = sq4[:, 1, :]
            sqf = sq4[:, 2, :]

            d_ = T1[:, 0, :]
            e_ = T1[:, 1, :]
            f_ = T1[:, 2, :]
            aq = T1[:, 3, :]
            cq = T2[:, 0, :]
            bq = T2[:, 1, :]

            # tiles
            DD = T("DD", [P, 2, PK])          # [df | de]
            bcp = T("bcp")
            defp = T("defp")
            tmp2 = T("tmp2", [P, 2, PK])
            s12 = T("s12")
            p2 = T("p2")
            p2c = T("p2c")
            bc2 = T("bc2", [P, 2, PK])        # [cq*sqd | bq*sqe]
            s2d = T("s2d")
            bmf = T("bmf")
            abf = T("abf")
            t1t = T("t1t")
            t2t = T("t2t")
            lnp = T("lnp", dt=F32)
            ipd = T("ipd")
            tp = T("tp")
            r0 = T("r0")
            r = T("r")
            lp = T("lp")
            lm = T("lm")
            dlm = T("dlm")
            arg = T("arg")
            at = T("at")
            c1 = T("c1")
            c3n = T("c3n")
            pc1 = T("pc1")
            pc3n = T("pc3n")
            AB = T("AB", [P, 2, PK])          # [b1 | a1]
            m24 = T("m24", [P, 2, PK])        # [e*b1 | f*a1]
            m5 = T("m5")
            wv = T("wv", [P, 3, PK])
            sww = T("sww", [P, 3, PK])
            n12 = T("n12")
            nrm = T("nrm")

            # ---- DVE phase A per half (interleaved for pipelining) ----
            def early(hs):
                nc.vector.tensor_mul(out=DD[:, 1, hs], in0=d_[:, hs],
                                     in1=e_[:, hs])
                nc.vector.tensor_mul(out=DD[:, 0, hs], in0=d_[:, hs],
                                     in1=f_[:, hs])
                nc.vector.tensor_mul(out=bcp[:, hs], in0=cq[:, hs],
                                     in1=bq[:, hs])
                nc.vector.tensor_mul(out=defp[:, hs], in0=DD[:, 1, hs],
                                     in1=f_[:, hs])

            def schain(hs):
                # tmp2 = [sqd+sqf, sqe+sqa]; s12; p2 = s12 - bcp; clamp
                nc.vector.tensor_add(out=tmp2[:, :, hs],
                                     in0=sq4[:, 0:2, hs],
                                     in1=sq4[:, 2:4, hs])
                nc.vector.tensor_add(out=s12[:, hs], in0=tmp2[:, 0, hs],
                                     in1=tmp2[:, 1, hs])
                nc.vector.tensor_sub(out=p2[:, hs], in0=s12[:, hs],
                                     in1=bcp[:, hs])
                nc.vector.tensor_scalar_max(out=p2c[:, hs], in0=p2[:, hs],
                                            scalar1=5e-6)

            def detchain(hs):
                # det = aq(bc - f^2) + 2def - (bq e^2 + cq d^2)
                nc.vector.tensor_mul(out=bc2[:, :, hs], in0=T2[:, :, hs],
                                     in1=sq4[:, 0:2, hs])
                nc.vector.tensor_add(out=s2d[:, hs], in0=bc2[:, 0, hs],
                                     in1=bc2[:, 1, hs])
                nc.vector.tensor_sub(out=bmf[:, hs], in0=bcp[:, hs],
                                     in1=sqf[:, hs])
                nc.vector.tensor_mul(out=abf[:, hs], in0=aq[:, hs],
                                     in1=bmf[:, hs])
                nc.vector.scalar_tensor_tensor(
                    out=t1t[:, hs], in0=defp[:, hs], scalar=2.0,
                    in1=abf[:, hs], op0=OP.mult, op1=OP.add)
                nc.vector.tensor_sub(out=t2t[:, hs], in0=t1t[:, hs],
                                     in1=s2d[:, hs])

            def ladder1(hs):
                nc.scalar.activation(out=lnp[:, hs], in_=p2c[:, hs],
                                     func=AF.Ln, scale=4.0 / 3.0)
                nc.scalar.activation(out=ipd[:, hs], in_=lnp[:, hs],
                                     func=AF.Exp, scale=-1.5, bias=ln4c)
                nc.scalar.activation(out=tp[:, hs], in_=lnp[:, hs],
                                     func=AF.Exp, scale=0.5)

            def rchain(hs):
                nc.vector.tensor_mul(out=r0[:, hs], in0=t2t[:, hs],
                                     in1=ipd[:, hs])
                nc.vector.tensor_scalar(out=r[:, hs], in0=r0[:, hs],
                                        scalar1=CLAMP, scalar2=-CLAMP,
                                        op0=OP.min, op1=OP.max)

            def ladder2(hs):
                nc.scalar.activation(out=lp[:, hs], in_=r[:, hs],
                                     func=AF.Ln, scale=0.5, bias=c05)
                nc.scalar.activation(out=lm[:, hs], in_=r[:, hs],
                                     func=AF.Ln, scale=-0.5, bias=c05)

            def argact(hs):
                return nc.scalar.activation(out=arg[:, hs], in_=dlm[:, hs],
                                            func=AF.Exp, scale=0.5)

            # emission order: ACT ladder of half k overlaps DVE det of k+1
            early(HH[0])
            early(HH[1])
            schain(HH[0])
            ladder1(HH[0])
            detchain(HH[0])
            schain(HH[1])
            ladder1(HH[1])
            detchain(HH[1])
            rchain(HH[0])
            ladder2(HH[0])
            rchain(HH[1])
            nc.vector.tensor_sub(out=dlm[:, HH[0]], in0=lm[:, HH[0]],
                                 in1=lp[:, HH[0]])
            a_arg0 = argact(HH[0])
            ladder2(HH[1])
            nc.vector.tensor_sub(out=dlm[:, HH[1]], in0=lm[:, HH[1]],
                                 in1=lp[:, HH[1]])
            a_arg = argact(HH[1])

            tl_b = tload(TBL_TRIG, "tl_trig")
            add_dep_helper(tl_b, a_arg.ins, False, "trig after exp")

            # ---- trig + eigvec chain per half ----
            for hs in HH:
                a_at = nc.scalar.activation(out=at[:, hs], in_=arg[:, hs],
                                            func=AF.Arctan)
                add_dep_helper(a_at.ins, tl_b, False, "at after trig load")
                nc.scalar.activation(out=c1[:, hs], in_=at[:, hs],
                                     func=AF.Sin, scale=-2.0 / 3.0,
                                     bias=pi2c)
                nc.scalar.activation(out=c3n[:, hs], in_=at[:, hs],
                                     func=AF.Sin, scale=-2.0 / 3.0,
                                     bias=mpi6c)

            def wchain(hs):
                nc.vector.tensor_mul(out=pc1[:, hs], in0=tp[:, hs],
                                     in1=c1[:, hs])
                nc.vector.tensor_mul(out=pc3n[:, hs], in0=tp[:, hs],
                                     in1=c3n[:, hs])
                nc.vector.tensor_sub(out=AB[:, 1, hs], in0=aq[:, hs],
                                     in1=pc1[:, hs])
                nc.vector.tensor_sub(out=AB[:, 0, hs], in0=bq[:, hs],
                                     in1=pc1[:, hs])
                # m24 = [e*b1, f*a1]
                nc.vector.tensor_mul(out=m24[:, :, hs], in0=T1[:, 1:3, hs],
                                     in1=AB[:, :, hs])
                nc.vector.tensor_mul(out=m5[:, hs], in0=AB[:, 1, hs],
                                     in1=AB[:, 0, hs])
                # wv[0:2] = [df - m2, de - m4]
                nc.vector.tensor_sub(out=wv[:, 0:2, hs], in0=DD[:, :, hs],
                                     in1=m24[:, :, hs])
                nc.vector.tensor_sub(out=wv[:, 2, hs], in0=m5[:, hs],
                                     in1=sqd[:, hs])

            def swwact(hs):
                nc.scalar.activation(out=sww[:, :, hs], in_=wv[:, :, hs],
                                     func=AF.Square)

            def nchain(hs):
                nc.vector.tensor_add(out=n12[:, hs], in0=sww[:, 0, hs],
                                     in1=sww[:, 1, hs])
                nc.vector.tensor_add(out=nrm[:, hs], in0=n12[:, hs],
                                     in1=sww[:, 2, hs])

            wchain(HH[0])
            swwact(HH[0])
            wchain(HH[1])
            swwact(HH[1])
            nchain(HH[0])
            nchain(HH[1])

            # ---- gpsimd branch: eigenvalue diffs (pc1/pc3n ready) ----
            dpc1 = T("dpc1", [P, CW])
            dpc3n = T("dpc3n", [P, CW])
            tsum = T("tsum", [P, CW])
            nc.gpsimd.tensor_sub(out=dpc1, in0=pc1[:, HH[0]],
                                 in1=pc1[:, HH[1]])
            nc.gpsimd.tensor_sub(out=dpc3n, in0=pc3n[:, HH[0]],
                                 in1=pc3n[:, HH[1]])
            nc.gpsimd.tensor_add(out=tsum, in0=dpc1, in1=dpc3n)

            # ---- tail: dot, norms, quake rsqrt, accumulations ----
            ds = T("ds", [P, 3, CW])
            nc.vector.tensor_mul(out=ds, in0=wv[:, :, HH[0]],
                                 in1=wv[:, :, HH[1]])
            d12 = T("d12", [P, CW])
            nc.vector.tensor_add(out=d12, in0=ds[:, 0, :], in1=ds[:, 1, :])
            dotv = T("dotv", [P, CW])
            nc.vector.tensor_add(out=dotv, in0=d12, in1=ds[:, 2, :])
            adot = T("adot", [P, CW])
            nc.vector.scalar_tensor_tensor(
                out=adot, in0=dotv, scalar=-1.0, in1=dotv,
                op0=OP.mult, op1=OP.max)

            nn0 = T("nn0", [P, CW])
            nc.vector.tensor_mul(out=nn0, in0=nrm[:, HH[0]],
                                 in1=nrm[:, HH[1]])
            junk = T("junk", [P, CW])
            if QUAKE:
                nnc = T("nnc", [P, CW])
                nc.vector.tensor_scalar_max(out=nnc, in0=nn0, scalar1=1e-30)
                # quake rsqrt (bf16): y0 = bits(0x5f37 - (i >> 1))
                i1 = T("i1", [P, CW], dt=U16)
                nc.vector.tensor_scalar(out=i1, in0=nnc.bitcast(U16),
                                        scalar1=1, scalar2=None,
                                        op0=OP.logical_shift_right)
                y0 = T("y0", [P, CW])
                nc.vector.tensor_scalar(out=y0.bitcast(U16), in0=i1,
                                        scalar1=MAGIC, scalar2=-1.0,
                                        op0=OP.subtract, op1=OP.mult)
                qm1 = T("qm1", [P, CW])
                nc.vector.tensor_mul(out=qm1, in0=nnc, in1=y0)
                qm2 = T("qm2", [P, CW])
                nc.vector.tensor_mul(out=qm2, in0=qm1, in1=y0)
                qnr = T("qnr", [P, CW])
                nc.vector.tensor_scalar(out=qnr, in0=qm2, scalar1=-0.5,
                                        scalar2=1.5, op0=OP.mult, op1=OP.add)
                yq = T("yq", [P, CW])
                nc.vector.tensor_mul(out=yq, in0=y0, in1=qnr)
                nc.vector.tensor_tensor_reduce(
                    out=junk, in0=adot, in1=yq, scale=1.0, scalar=0.0,
                    op0=OP.mult, op1=OP.add, accum_out=out_sb[:, 1:2])
            else:
                # baseline-proven tail: fp32 recip + ACT sqrt (3rd table)
                nn32 = T("nn32", [P, CW], dt=F32)
                nc.vector.tensor_mul(out=nn32, in0=nrm[:, HH[0]],
                                     in1=nrm[:, HH[1]])
                nnc = T("nnc", [P, CW], dt=F32)
                nc.vector.tensor_scalar_max(out=nnc, in0=nn32,
                                            scalar1=1e-30)
                inn = T("inn", [P, CW], dt=F32)
                nc.vector.reciprocal_approx_fast(out=inn, in_=nnc)
                tl_c = tload(TBL_SQRT, "tl_sqrt")
                rn = T("rn", [P, CW])
                a_rn = nc.scalar.activation(out=rn, in_=inn, func=AF.Sqrt)
                add_dep_helper(a_rn.ins, tl_c, False, "rn after sqrt load")
                nc.vector.scalar_tensor_tensor(
                    out=junk, in0=adot, scalar=1.0, in1=rn,
                    op0=OP.mult, op1=OP.mult,
                    accum_out=out_sb[:, 1:2])

            # ---- eigenvalue-diff tail (dq space) ----
            dl3w = T("dl3w", [P, 3, CW])
            nc.vector.tensor_add(out=dl3w[:, 0, :], in0=dpc1, in1=dqt)
            nc.vector.tensor_add(out=dl3w[:, 1, :], in0=dpc3n, in1=dqt)
            nc.vector.tensor_sub(out=dl3w[:, 2, :], in0=dqt, in1=tsum)
            dabs = T("dabs", [P, 3, CW])
            nc.vector.scalar_tensor_tensor(
                out=dabs, in0=dl3w, scalar=-1.0, in1=dl3w,
                op0=OP.mult, op1=OP.max, accum_out=out_sb[:, 0:1])

            nc.sync.dma_start(out=out[:, :], in_=out_sb)
    nc.finalize()
    return nc


_NC = None


def _get_nc():
    global _NC
    if _NC is None:
        _NC = _build()
    return _NC


def _shard_inputs(input_data, target, mask):
    """Full inputs -> per-core in_maps: bf16 packed channel planes
    [d,e,f,aq,cq,bq] + dq with benign diag(1,2,3) pad slots."""
    x = np.asarray(input_data, dtype=np.float32)
    t = np.asarray(target, dtype=np.float32)
    m = np.asarray(mask)
    in_maps = []
    total_pads = 0
    cap = P * CW

    def chans(slab):
        # slab [6, N] with channel order a,d,e,b,f,c
        a, d, e, b, f, c = slab
        q = (a + b + c) * (1.0 / 3.0)
        return np.stack([d, e, f, a - q, c - q, b - q]), q

    for k in range(NCORES):
        bidx = k // (NCORES // B)
        h0 = HS * (k % (NCORES // B))
        xs, qi = chans(x[bidx, :, h0:h0 + HS].reshape(C, -1))
        ts_, qt = chans(t[bidx, :, h0:h0 + HS].reshape(C, -1))
        mb = (m[bidx, 0, 0, h0:h0 + HS].reshape(-1) == 1)
        pos = np.flatnonzero(mb)
        ncnt = pos.size
        if ncnt > cap:
            raise _CapacityError(
                f"masked count {ncnt} exceeds capacity {cap}")
        total_pads += cap - ncnt
        gin = np.empty((6, cap), np.float32)
        gtg = np.empty((6, cap), np.float32)
        gin[:, :ncnt] = xs[:, pos]
        gtg[:, :ncnt] = ts_[:, pos]
        for ci in range(6):
            gin[ci, ncnt:] = PAD_CH[ci]
            gtg[ci, ncnt:] = PAD_CH[ci]
        dq = np.zeros(cap, np.float32)
        dq[:ncnt] = qi[pos] - qt[pos]
        xg = np.empty((6, P, PK), np.float32)
        xg[:, :, :CW] = gin.reshape(6, P, CW)
        xg[:, :, CW:] = gtg.reshape(6, P, CW)
        in_maps.append({
            "x": np.ascontiguousarray(xg.astype(ml_dtypes.bfloat16)),
            "dqx": np.ascontiguousarray(
                dq.reshape(P, CW).astype(ml_dtypes.bfloat16)),
        })
    return in_maps, total_pads


def _host_reference(input_data, target, mask):
    """Exact numpy fallback (only if a mask ever exceeds the compact
    capacity, which cannot happen for the advertised input statistics)."""
    idx = np.array([[0, 1, 2], [1, 3, 4], [2, 4, 5]])

    def sym(t):
        return np.moveaxis(t, 1, -1)[..., idx]

    m = (np.asarray(mask)[:, 0, 0] == 1)
    mf = m.astype(np.float64)
    cntv = mf.sum()
    wi, vi = np.linalg.eigh(sym(np.asarray(input_data, np.float64)))
    wt, vt = np.linalg.eigh(sym(np.asarray(target, np.float64)))
    val = (np.abs(wi - wt).sum(-1) * mf).sum() / (3.0 * cntv)
    dot = np.abs((vi[..., :, 2] * vt[..., :, 2]).sum(-1))
    vec = 1.0 - (dot * mf).sum() / cntv
    return (np.float32(val), np.float32(vec))


def kernel(input_data, target, mask, root_dir=0, _trace=False):
    nc = _get_nc()
    try:
        in_maps, total_pads = _shard_inputs(
            np.asarray(input_data), np.asarray(target), np.asarray(mask))
    except _CapacityError:
        return _host_reference(input_data, target, mask)
    res = run_bass_kernel_spmd(nc, in_maps, core_ids=list(range(NCORES)),
                               trace=_trace)
    outs = res.results
    val_sum = 0.0
    dot_sum = 0.0
    for om in outs:
        o = om["out"].astype(np.float64)
        val_sum += o[:, 0].sum()
        dot_sum += o[:, 1].sum()
    dot_sum -= total_pads          # each pad contributes exactly |cos| = 1
    cnt = float((np.asarray(mask)[:, 0, 0] == 1).sum())
    val_loss = np.float32(val_sum / (3.0 * cnt))
    vec_loss = np.float32(1.0 - dot_sum / cnt)
    if _trace:
        return (val_loss, vec_loss), res
    return (val_loss, vec_loss)


# revision 10
# speedup vs baseline: 1.1305x; 1.1305x over previous
"""Trainium2 Bass kernel for the eigenvalue/eigenvector loss
(nn_AV_loss): per-voxel 3x3 symmetric eigendecomposition of input and
target tensors, masked L1 of sorted eigenvalues + masked principal-axis
|cosine|, reduced to two scalars.

Self-contained: hardcodes shapes/sharding. kernel(**inputs) takes FULL
inputs and returns the full output (val_loss, vec_loss).

Sharding: fully data-parallel over B*H (2*80 = 160 -> 20 H-slices per
core); per-core partial masked sums are returned and reduced on host.

v3 design (from the 54.5us v2):
- 6 device channels [d,e,f,aq] + [cq,bq] (q no longer shipped; host
  ships dq = q_in - q_tgt as a half plane) in bf16, halved DMA pieces
  so compute starts ~2us earlier.
- p2 via the zero-trace identity with aq^2 from the ACT square batch:
  p2' = (d^2+e^2+f^2+aq^2) - bq*cq  (kills 3 DVE adds).
- merged 2-wide DVE ops (s-pairs, bq*sqe|cq*sqd, e*b1|f*a1,
  df-m2|de-m4) to amortize per-op overhead.
- eigenvalue-diff tail in difference space: dl1 = dpc1+dq etc., abs via
  tensor_scalar(abs_max, 0) with free accum (no ACT Abs, no q planes).
- bf16 "Quake" rsqrt on DVE (2 uint16 tensor_scalar + 1 Newton step)
  for the eigvec normalization: kills the third ACT table load and the
  DVE iterative reciprocal.
- gpsimd (Pool) takes the dpc1/dpc3n/tsum branch (otherwise idle).
- input|target half-split software pipelining through the whole spine
  so ACT ladder overlaps the DVE det chain of the other half.
- 2 ACT table loads total: natural_log_exp -> trig_and_small.

Masked-voxel compaction: host packs ALL masked voxels of a core
row-major into [128, CW] (pad slots get a benign diag(1,2,3) matrix
that adds exactly 0 to the eigenvalue-L1 sum and exactly 1 per pad to
the |dot| sum, subtracted on host).
"""

import numpy as np
import ml_dtypes

import concourse.tile as tile
from concourse import mybir
from concourse.bacc import Bacc
from concourse.bass_utils import run_bass_kernel_spmd
from bass_rust import add_dep_helper


class _CapacityError(RuntimeError):
    pass


AF = mybir.ActivationFunctionType
OP = mybir.AluOpType
F32 = mybir.dt.float32
BF16 = mybir.dt.bfloat16
U16 = mybir.dt.uint16

NCORES = 8
B, C, H, W, D = 2, 6, 80, 80, 80
HS = H // (NCORES // B)          # 20 h-slices per core
P = 128
CW = 504                         # compact width (max masked 64269 <= 64512)
PK = 2 * CW                      # packed cols: [input | target]

TBL_LNEXP = 6                    # natural_log_exp_and_others
TBL_TRIG = 9                     # trig_and_small (arctan + sin)
TBL_SQRT = 3                     # sqrt_and_others (fallback tail only)
QUAKE = False                     # DVE quake-rsqrt tail vs ACT sqrt tail

# benign pad matrix diag(1,2,3): q=2, aq=-1, bq=0, cq=1, d=e=f=0
# lam={3,2,1}, input==target so d|lam|=0 and |cos|=1 per pad
PAD_CH = (0.0, 0.0, 0.0, -1.0, 1.0, 0.0)   # d,e,f,aq,cq,bq

CLAMP = 1.0 - 3e-7
PI2 = float(np.pi / 2.0)
MPI6 = float(-np.pi / 6.0)
LN4 = float(np.log(4.0))
MAGIC = float(0x5F37)            # bf16 quake-rsqrt magic (top16 of 0x5f3759df)


def _build():
    nc = Bacc()
    x = nc.dram_tensor("x", [6, P, PK], BF16, kind="ExternalInput")
    dqx = nc.dram_tensor("dqx", [P, CW], BF16, kind="ExternalInput")
    out = nc.dram_tensor("out", [P, 2], F32, kind="ExternalOutput")

    def tload(set_id, name):
        raw = mybir.InstLoadActFuncSet(
            name=name, ins=[], outs=[], act_func_set_id=set_id)
        nc.scalar.add_instruction(raw)
        return raw

    HH = (slice(0, CW), slice(CW, PK))

    with tile.TileContext(nc) as tc:
        with tc.tile_pool(name="main", bufs=1) as pool:

            def T(tag, shape=None, dt=BF16):
                return pool.tile(shape or [P, PK], dt, tag=tag, name=tag)

            out_sb = pool.tile([P, 2], F32, tag="out_sb")
            c05 = pool.tile([P, 1], F32, tag="c05")
            nc.vector.memset(c05, 0.5)
            pi2c = pool.tile([P, 1], F32, tag="pi2c")
            nc.vector.memset(pi2c, PI2)
            mpi6c = pool.tile([P, 1], F32, tag="mpi6c")
            nc.vector.memset(mpi6c, MPI6)
            ln4c = pool.tile([P, 1], F32, tag="ln4c")
            nc.vector.memset(ln4c, LN4)

            tl_a = tload(TBL_LNEXP, "tl_lnexp")

            # ---- loads: T1 = [d,e,f,aq], T2 = [cq,bq], dq ----
            T1 = T("T1", [P, 4, PK])
            T2 = T("T2", [P, 2, PK])
            dqt = T("dqt", [P, CW])
            for hs in HH:
                for i in range(4):
                    nc.sync.dma_start(out=T1[:, i, hs], in_=x[i, :, hs])
                for i in range(2):
                    nc.sync.dma_start(out=T2[:, i, hs], in_=x[4 + i, :, hs])
            nc.sync.dma_start(out=dqt, in_=dqx[:, :])

            # ---- ACT: squares of [d,e,f,aq] per half ----
            sq4 = T("sq4", [P, 4, PK])
            for hs in HH:
                a_sq = nc.scalar.activation(out=sq4[:, :, hs],
                                            in_=T1[:, :, hs],
                                            func=AF.Square)
                add_dep_helper(a_sq.ins, tl_a, False, "sq after tbl")
            sqd = sq4[:, 0, :]
            sqe = sq4[:, 1, :]
            sqf = sq4[:, 2, :]

            d_ = T1[:, 0, :]
            e_ = T1[:, 1, :]
            f_ = T1[:, 2, :]
            aq = T1[:, 3, :]
            cq = T2[:, 0, :]
            bq = T2[:, 1, :]

            # tiles
            DD = T("DD", [P, 2, PK])          # [df | de]
            bcp = T("bcp")
            defp = T("defp")
            tmp2 = T("tmp2", [P, 2, PK])
            s12 = T("s12")
            p2 = T("p2")
            p2c = T("p2c")
            bc2 = T("bc2", [P, 2, PK])        # [cq*sqd | bq*sqe]
            s2d = T("s2d")
            bmf = T("bmf")
            abf = T("abf")
            t1t = T("t1t")
            t2t = T("t2t")
            lnp = T("lnp", dt=F32)
            ipd = T("ipd")
            tp = T("tp")
            r0 = T("r0")
            r = T("r")
            lp = T("lp")
            lm = T("lm")
            dlm = T("dlm")
            arg = T("arg")
            at = T("at")
            c1 = T("c1")
            c3n = T("c3n")
            pc1 = T("pc1")
            pc3n = T("pc3n")
            AB = T("AB", [P, 2, PK])          # [b1 | a1]
            m24 = T("m24", [P, 2, PK])        # [e*b1 | f*a1]
            m5 = T("m5")
            wv = T("wv", [P, 3, PK])
            sww = T("sww", [P, 3, PK])
            n12 = T("n12")
            nrm = T("nrm")

            # ---- DVE phase A per half (interleaved for pipelining) ----
            def early(hs):
                nc.vector.tensor_mul(out=DD[:, 1, hs], in0=d_[:, hs],
                                     in1=e_[:, hs])
                nc.vector.tensor_mul(out=DD[:, 0, hs], in0=d_[:, hs],
                                     in1=f_[:, hs])
                nc.vector.tensor_mul(out=bcp[:, hs], in0=cq[:, hs],
                                     in1=bq[:, hs])
                nc.vector.tensor_mul(out=defp[:, hs], in0=DD[:, 1, hs],
                                     in1=f_[:, hs])

            def schain(hs):
                # tmp2 = [sqd+sqf, sqe+sqa]; s12; p2 = s12 - bcp; clamp
                nc.vector.tensor_add(out=tmp2[:, :, hs],
                                     in0=sq4[:, 0:2, hs],
                                     in1=sq4[:, 2:4, hs])
                nc.vector.tensor_add(out=s12[:, hs], in0=tmp2[:, 0, hs],
                                     in1=tmp2[:, 1, hs])
                nc.vector.tensor_sub(out=p2[:, hs], in0=s12[:, hs],
                                     in1=bcp[:, hs])
                nc.vector.tensor_scalar_max(out=p2c[:, hs], in0=p2[:, hs],
                                            scalar1=5e-6)

            def detchain(hs):
                # det = aq(bc - f^2) + 2def - (bq e^2 + cq d^2)
                nc.vector.tensor_mul(out=bc2[:, :, hs], in0=T2[:, :, hs],
                                     in1=sq4[:, 0:2, hs])
                nc.vector.tensor_add(out=s2d[:, hs], in0=bc2[:, 0, hs],
                                     in1=bc2[:, 1, hs])
                nc.vector.tensor_sub(out=bmf[:, hs], in0=bcp[:, hs],
                                     in1=sqf[:, hs])
                nc.vector.tensor_mul(out=abf[:, hs], in0=aq[:, hs],
                                     in1=bmf[:, hs])
                nc.vector.scalar_tensor_tensor(
                    out=t1t[:, hs], in0=defp[:, hs], scalar=2.0,
                    in1=abf[:, hs], op0=OP.mult, op1=OP.add)
                nc.vector.tensor_sub(out=t2t[:, hs], in0=t1t[:, hs],
                                     in1=s2d[:, hs])

            def ladder1(hs):
                nc.scalar.activation(out=lnp[:, hs], in_=p2c[:, hs],
                                     func=AF.Ln, scale=4.0 / 3.0)
                nc.scalar.activation(out=ipd[:, hs], in_=lnp[:, hs],
                                     func=AF.Exp, scale=-1.5, bias=ln4c)
                nc.scalar.activation(out=tp[:, hs], in_=lnp[:, hs],
                                     func=AF.Exp, scale=0.5)

            def rchain(hs):
                nc.vector.tensor_mul(out=r0[:, hs], in0=t2t[:, hs],
                                     in1=ipd[:, hs])
                nc.vector.tensor_scalar(out=r[:, hs], in0=r0[:, hs],
                                        scalar1=CLAMP, scalar2=-CLAMP,
                                        op0=OP.min, op1=OP.max)

            def ladder2(hs):
                nc.scalar.activation(out=lp[:, hs], in_=r[:, hs],
                                     func=AF.Ln, scale=0.5, bias=c05)
                nc.scalar.activation(out=lm[:, hs], in_=r[:, hs],
                                     func=AF.Ln, scale=-0.5, bias=c05)

            def argact(hs):
                return nc.scalar.activation(out=arg[:, hs], in_=dlm[:, hs],
                                            func=AF.Exp, scale=0.5)

            # emission order: ACT ladder of half k overlaps DVE det of k+1
            early(HH[0])
            early(HH[1])
            schain(HH[0])
            ladder1(HH[0])
            detchain(HH[0])
            schain(HH[1])
            ladder1(HH[1])
            detchain(HH[1])
            rchain(HH[0])
            ladder2(HH[0])
            rchain(HH[1])
            nc.vector.tensor_sub(out=dlm[:, HH[0]], in0=lm[:, HH[0]],
                                 in1=lp[:, HH[0]])
            a_arg0 = argact(HH[0])
            ladder2(HH[1])
            nc.vector.tensor_sub(out=dlm[:, HH[1]], in0=lm[:, HH[1]],
                                 in1=lp[:, HH[1]])
            a_arg = argact(HH[1])

            tl_b = tload(TBL_TRIG, "tl_trig")
            add_dep_helper(tl_b, a_arg.ins, False, "trig after exp")

            # ---- trig + eigvec chain per half ----
            for hs in HH:
                a_at = nc.scalar.activation(out=at[:, hs], in_=arg[:, hs],
                                            func=AF.Arctan)
                add_dep_helper(a_at.ins, tl_b, False, "at after trig load")
                nc.scalar.activation(out=c1[:, hs], in_=at[:, hs],
                                     func=AF.Sin, scale=-2.0 / 3.0,
                                     bias=pi2c)
                nc.scalar.activation(out=c3n[:, hs], in_=at[:, hs],
                                     func=AF.Sin, scale=-2.0 / 3.0,
                                     bias=mpi6c)

            def wchain(hs):
                nc.vector.tensor_mul(out=pc1[:, hs], in0=tp[:, hs],
                                     in1=c1[:, hs])
                nc.vector.tensor_mul(out=pc3n[:, hs], in0=tp[:, hs],
                                     in1=c3n[:, hs])
                nc.vector.tensor_sub(out=AB[:, 1, hs], in0=aq[:, hs],
                                     in1=pc1[:, hs])
                nc.vector.tensor_sub(out=AB[:, 0, hs], in0=bq[:, hs],
                                     in1=pc1[:, hs])
                # m24 = [e*b1, f*a1]
                nc.vector.tensor_mul(out=m24[:, :, hs], in0=T1[:, 1:3, hs],
                                     in1=AB[:, :, hs])
                nc.vector.tensor_mul(out=m5[:, hs], in0=AB[:, 1, hs],
                                     in1=AB[:, 0, hs])
                # wv[0:2] = [df - m2, de - m4]
                nc.vector.tensor_sub(out=wv[:, 0:2, hs], in0=DD[:, :, hs],
                                     in1=m24[:, :, hs])
                nc.vector.tensor_sub(out=wv[:, 2, hs], in0=m5[:, hs],
                                     in1=sqd[:, hs])

            def swwact(hs):
                return nc.scalar.activation(out=sww[:, :, hs],
                                            in_=wv[:, :, hs],
                                            func=AF.Square)

            def nchain(hs):
                nc.vector.tensor_add(out=n12[:, hs], in0=sww[:, 0, hs],
                                     in1=sww[:, 1, hs])
                nc.vector.tensor_add(out=nrm[:, hs], in0=n12[:, hs],
                                     in1=sww[:, 2, hs])

            wchain(HH[0])
            swwact(HH[0])
            wchain(HH[1])
            a_sww1 = swwact(HH[1])
            nchain(HH[0])
            nchain(HH[1])

            # ---- gpsimd branch: eigenvalue diffs (pc1/pc3n ready) ----
            dpc1 = T("dpc1", [P, CW])
            dpc3n = T("dpc3n", [P, CW])
            tsum = T("tsum", [P, CW])
            nc.gpsimd.tensor_sub(out=dpc1, in0=pc1[:, HH[0]],
                                 in1=pc1[:, HH[1]])
            nc.gpsimd.tensor_sub(out=dpc3n, in0=pc3n[:, HH[0]],
                                 in1=pc3n[:, HH[1]])
            nc.gpsimd.tensor_add(out=tsum, in0=dpc1, in1=dpc3n)

            # ---- tail: dot, norms, quake rsqrt, accumulations ----
            ds = T("ds", [P, 3, CW])
            nc.vector.tensor_mul(out=ds, in0=wv[:, :, HH[0]],
                                 in1=wv[:, :, HH[1]])
            d12 = T("d12", [P, CW])
            nc.vector.tensor_add(out=d12, in0=ds[:, 0, :], in1=ds[:, 1, :])
            dotv = T("dotv", [P, CW])
            nc.vector.tensor_add(out=dotv, in0=d12, in1=ds[:, 2, :])
            adot = T("adot", [P, CW])
            nc.vector.scalar_tensor_tensor(
                out=adot, in0=dotv, scalar=-1.0, in1=dotv,
                op0=OP.mult, op1=OP.max)

            nn0 = T("nn0", [P, CW])
            nc.vector.tensor_mul(out=nn0, in0=nrm[:, HH[0]],
                                 in1=nrm[:, HH[1]])
            junk = T("junk", [P, CW])
            if QUAKE:
                nnc = T("nnc", [P, CW])
                nc.vector.tensor_scalar_max(out=nnc, in0=nn0, scalar1=1e-30)
                # quake rsqrt (bf16): y0 = bits(0x5f37 - (i >> 1))
                i1 = T("i1", [P, CW], dt=U16)
                nc.vector.tensor_scalar(out=i1, in0=nnc.bitcast(U16),
                                        scalar1=1, scalar2=None,
                                        op0=OP.logical_shift_right)
                y0 = T("y0", [P, CW])
                nc.vector.tensor_scalar(out=y0.bitcast(U16), in0=i1,
                                        scalar1=MAGIC, scalar2=-1.0,
                                        op0=OP.subtract, op1=OP.mult)
                qm1 = T("qm1", [P, CW])
                nc.vector.tensor_mul(out=qm1, in0=nnc, in1=y0)
                qm2 = T("qm2", [P, CW])
                nc.vector.tensor_mul(out=qm2, in0=qm1, in1=y0)
                qnr = T("qnr", [P, CW])
                nc.vector.tensor_scalar(out=qnr, in0=qm2, scalar1=-0.5,
                                        scalar2=1.5, op0=OP.mult, op1=OP.add)
                yq = T("yq", [P, CW])
                nc.vector.tensor_mul(out=yq, in0=y0, in1=qnr)
                nc.vector.tensor_tensor_reduce(
                    out=junk, in0=adot, in1=yq, scale=1.0, scalar=0.0,
                    op0=OP.mult, op1=OP.add, accum_out=out_sb[:, 1:2])
            else:
                # baseline-proven tail: fp32 recip + ACT sqrt (3rd table)
                nn32 = T("nn32", [P, CW], dt=F32)
                nc.vector.tensor_mul(out=nn32, in0=nrm[:, HH[0]],
                                     in1=nrm[:, HH[1]])
                nnc = T("nnc", [P, CW], dt=F32)
                nc.vector.tensor_scalar_max(out=nnc, in0=nn32,
                                            scalar1=1e-30)
                inn = T("inn", [P, CW], dt=F32)
                nc.vector.reciprocal_approx_fast(out=inn, in_=nnc)
                tl_c = tload(TBL_SQRT, "tl_sqrt")
                # pin the sqrt table load AFTER the last trig-table user,
                # else the scheduler floats it to the top and walrus
                # thrashes table loads through the whole ln/exp ladder
                add_dep_helper(tl_c, a_sww1.ins, False, "tbl sqrt last")
                rn = T("rn", [P, CW])
                a_rn = nc.scalar.activation(out=rn, in_=inn, func=AF.Sqrt)
                add_dep_helper(a_rn.ins, tl_c, False, "rn after sqrt load")
                nc.vector.scalar_tensor_tensor(
                    out=junk, in0=adot, scalar=1.0, in1=rn,
                    op0=OP.mult, op1=OP.mult,
                    accum_out=out_sb[:, 1:2])

            # ---- eigenvalue-diff tail (dq space) ----
            dl3w = T("dl3w", [P, 3, CW])
            nc.vector.tensor_add(out=dl3w[:, 0, :], in0=dpc1, in1=dqt)
            nc.vector.tensor_add(out=dl3w[:, 1, :], in0=dpc3n, in1=dqt)
            nc.vector.tensor_sub(out=dl3w[:, 2, :], in0=dqt, in1=tsum)
            dabs = T("dabs", [P, 3, CW])
            nc.vector.scalar_tensor_tensor(
                out=dabs, in0=dl3w, scalar=-1.0, in1=dl3w,
                op0=OP.mult, op1=OP.max, accum_out=out_sb[:, 0:1])

            nc.sync.dma_start(out=out[:, :], in_=out_sb)
    nc.finalize()
    return nc


_NC = None


def _get_nc():
    global _NC
    if _NC is None:
        _NC = _build()
    return _NC


def _shard_inputs(input_data, target, mask):
    """Full inputs -> per-core in_maps: bf16 packed channel planes
    [d,e,f,aq,cq,bq] + dq with benign diag(1,2,3) pad slots."""
    x = np.asarray(input_data, dtype=np.float32)
    t = np.asarray(target, dtype=np.float32)
    m = np.asarray(mask)
    in_maps = []
    total_pads = 0
    cap = P * CW

    def chans(slab):
        # slab [6, N] with channel order a,d,e,b,f,c
        a, d, e, b, f, c = slab
        q = (a + b + c) * (1.0 / 3.0)
        return np.stack([d, e, f, a - q, c - q, b - q]), q

    for k in range(NCORES):
        bidx = k // (NCORES // B)
        h0 = HS * (k % (NCORES // B))
        xs, qi = chans(x[bidx, :, h0:h0 + HS].reshape(C, -1))
        ts_, qt = chans(t[bidx, :, h0:h0 + HS].reshape(C, -1))
        mb = (m[bidx, 0, 0, h0:h0 + HS].reshape(-1) == 1)
        pos = np.flatnonzero(mb)
        ncnt = pos.size
        if ncnt > cap:
            raise _CapacityError(
                f"masked count {ncnt} exceeds capacity {cap}")
        total_pads += cap - ncnt
        gin = np.empty((6, cap), np.float32)
        gtg = np.empty((6, cap), np.float32)
        gin[:, :ncnt] = xs[:, pos]
        gtg[:, :ncnt] = ts_[:, pos]
        for ci in range(6):
            gin[ci, ncnt:] = PAD_CH[ci]
            gtg[ci, ncnt:] = PAD_CH[ci]
        dq = np.zeros(cap, np.float32)
        dq[:ncnt] = qi[pos] - qt[pos]
        xg = np.empty((6, P, PK), np.float32)
        xg[:, :, :CW] = gin.reshape(6, P, CW)
        xg[:, :, CW:] = gtg.reshape(6, P, CW)
        in_maps.append({
            "x": np.ascontiguousarray(xg.astype(ml_dtypes.bfloat16)),
            "dqx": np.ascontiguousarray(
                dq.reshape(P, CW).astype(ml_dtypes.bfloat16)),
        })
    return in_maps, total_pads


def _host_reference(input_data, target, mask):
    """Exact numpy fallback (only if a mask ever exceeds the compact
    capacity, which cannot happen for the advertised input statistics)."""
    idx = np.array([[0, 1, 2], [1, 3, 4], [2, 4, 5]])

    def sym(t):
        return np.moveaxis(t, 1, -1)[..., idx]

    m = (np.asarray(mask)[:, 0, 0] == 1)
    mf = m.astype(np.float64)
    cntv = mf.sum()
    wi, vi = np.linalg.eigh(sym(np.asarray(input_data, np.float64)))
    wt, vt = np.linalg.eigh(sym(np.asarray(target, np.float64)))
    val = (np.abs(wi - wt).sum(-1) * mf).sum() / (3.0 * cntv)
    dot = np.abs((vi[..., :, 2] * vt[..., :, 2]).sum(-1))
    vec = 1.0 - (dot * mf).sum() / cntv
    return (np.float32(val), np.float32(vec))


def kernel(input_data, target, mask, root_dir=0, _trace=False):
    nc = _get_nc()
    try:
        in_maps, total_pads = _shard_inputs(
            np.asarray(input_data), np.asarray(target), np.asarray(mask))
    except _CapacityError:
        return _host_reference(input_data, target, mask)
    res = run_bass_kernel_spmd(nc, in_maps, core_ids=list(range(NCORES)),
                               trace=_trace)
    outs = res.results
    val_sum = 0.0
    dot_sum = 0.0
    for om in outs:
        o = om["out"].astype(np.float64)
        val_sum += o[:, 0].sum()
        dot_sum += o[:, 1].sum()
    dot_sum -= total_pads          # each pad contributes exactly |cos| = 1
    cnt = float((np.asarray(mask)[:, 0, 0] == 1).sum())
    val_loss = np.float32(val_sum / (3.0 * cnt))
    vec_loss = np.float32(1.0 - dot_sum / cnt)
    if _trace:
        return (val_loss, vec_loss), res
    return (val_loss, vec_loss)
